# revision 4
# baseline (speedup 1.0000x reference)
"""KMeans assignment kernel for Trainium2 (8 NeuronCores, data-parallel SPMD).

argmin_k ||f_n - c_k||^2  ==  argmax_k (2*f.c_k - |c_k|^2).  The score is
accumulated in ONE PSUM tile per row-tile, at scale 2^6:

    2^6 * s  =  fh16 @ C16s  +  a8 @ CL8 + b8 @ CH8  +  ones2 @ NL

  - main pass   : fh16 = f16(f) stationary, C16s = f16(2^6 * 2c) moving,
                  8 f16 streams (the PE's native 1 cyc/col rate).
  - corrections : ONE fp8e4 DoubleRow pass = 8 DR matmuls, each contracting
                  2x128 rows (fh-chunk paired with fl-chunk), computing
                  a8@CL8 + b8@CH8 together at ~1.9x the f16 MAC rate.
                  a8 = f8(fh16) pairs CL8 = f8(2^6*(2c - C16s/2^6));
                  b8 = f8(2^11*(f - fh16)) pairs CH8 = f8(2^-5 * 2c).
  - ncsq bias   : -2^6*|c|^2 as an f32 row, partition-broadcast once and
                  added on the DVE before the argmax (off the PE critical
                  path - moving it off the PE measured -65us).

Scheme error sigma ~2.7e-4 on the s-scale - same class as a 3-pass f16
hi/lo kernel; exact-match argmin vs the fp32 reference on the benchmark
input (verified on HW, and in host emulation: 0 flips, min margin 2.7e-4
dominated by one true near-tie row with fp64 gap 4e-4).

Structure per row-tile (software-pipelined one tile ahead on the PE):
  DMA(SP ring) ftile -> PE fp32 transpose (4 chunks, self-loading, into
  PSUM) -> ACT f16 copy-out (hi) -> DVE subtract (lo, f16) -> ACT fp8
  conversions (b side carries 2^11) -> PE matmul group (main + DR) ->
  DVE bias-add + max + max_index.  No DMA-xbar transposes (measured
  1.7us/tile on the queue - slower than the PE route), no DVE adds
  (tensor_tensor_reduce wedges this device; max/max_index only).

Sharding: features split over N across 8 cores; centroid-side operands
replicated (host-prepped into final SBUF layouts); no cross-core traffic.
Measured: ~570-600 us (host-dependent) vs the 849-868 us 3-pass-f16
baseline (~1.5x), exact output on every run.
"""
import sys

sys.path.insert(0, "/opt/trn_rl_repo")

import os
import numpy as np
import ml_dtypes
from contextlib import ExitStack, nullcontext

import concourse.bacc as bacc
import concourse.mybir as mybir
from concourse import tile
from concourse.masks import make_identity

N, D, K = 131072, 512, 1024
N_CORES = 8
N_PER_CORE = N // N_CORES          # 16384
N_TILES = N_PER_CORE // 128        # 128 row-tiles per core
ND = D // 128                      # 4 contraction chunks
F32 = mybir.dt.float32
F16 = mybir.dt.float16
F8 = mybir.dt.float8e4
U32 = mybir.dt.uint32
DRMODE = mybir.MatmulPerfMode.DoubleRow
COPY = mybir.ActivationFunctionType.Copy
E4NP = ml_dtypes.float8_e4m3
BSCALE = 2048.0                    # 2^11: b-side fp8 normalization
PSCALE = 64.0                      # 2^6: PSUM accumulation scale

_cached = {}


def _dedup_ldweights(nc):
    """Remove InstLdweights that reload weights already resident in the PE.

    The tile legalizer emits one Ldweights per non-self-loading matmul;
    consecutive matmuls sharing a stationary operand reload it redundantly.
    Runs after TileContext exit, before nc.finalize().  Any waits on a
    removed ld are moved to the next instruction.
    """
    removed = 0
    for fn in nc.m.functions:
        for b in fn.blocks:
            insts = b.instructions
            out = []
            loaded_key = None
            pending_waits = []
            for inst in insts:
                nm = type(inst).__name__
                if nm == "InstLdweights":
                    ap = inst.ins[0]
                    key = (str(ap.memref), ap.offset, str(ap.ap), str(ap.dtype))
                    if key == loaded_key:
                        si = inst.sync_info
                        if si is not None and len(si.on_wait) > 0:
                            pending_waits.extend(si.on_wait)
                        if si is not None and len(si.on_update) > 0:
                            out.append(inst)  # has side effects: keep
                            continue
                        removed += 1
                        continue
                    loaded_key = key
                elif nm in ("InstMatmult", "InstMatmultMx"):
                    wdt = inst.ins[1].dtype
                    if mybir.dt.size(wdt) == 4:
                        loaded_key = None  # self-loading matmul clobbers PE
                if pending_waits:
                    si = inst.sync_info
                    waits = list(si.on_wait) if si else []
                    ups = list(si.on_update) if si else []
                    inst.sync_info = mybir.SyncInfo(
                        on_wait=waits + pending_waits, on_update=ups)
                    pending_waits = []
                out.append(inst)
            if removed:
                b.instructions = out
    return removed


def build_bass(n_tiles: int = N_TILES, repeat: int = 1,
               mp_bufs: int = 3, psA_bufs: int = 2,
               work_bufs: int = 4, tps_bufs: int = 4,
               dedup: bool = True):
    n_rows = n_tiles * 128
    nc = bacc.Bacc()
    feat = nc.declare_dram_parameter("features", [n_rows, D], F32, isOutput=False)
    cmain_in = nc.declare_dram_parameter("cmain", [128, ND * K], F16, isOutput=False)
    cc8_in = nc.declare_dram_parameter("cc8", [128, ND * 2 * K], F8, isOutput=False)
    nrow_in = nc.declare_dram_parameter("nrow", [1, K], F32, isOutput=False)
    out = nc.declare_dram_parameter("out", [n_rows, 1], F32, isOutput=True)

    with tile.TileContext(nc) as tc, ExitStack() as ctx:
        const = ctx.enter_context(tc.tile_pool(name="const", bufs=1))
        work = ctx.enter_context(tc.tile_pool(name="work", bufs=work_bufs))
        tps = ctx.enter_context(tc.tile_pool(name="tps", bufs=tps_bufs))
        red = ctx.enter_context(tc.tile_pool(name="red", bufs=4))
        psA = ctx.enter_context(tc.tile_pool(name="psA", bufs=psA_bufs, space="PSUM"))
        psM = ctx.enter_context(tc.tile_pool(name="psM", bufs=mp_bufs, space="PSUM"))

        cmain = const.tile([128, ND * K], F16)
        nc.sync.dma_start(out=cmain[:], in_=cmain_in[:])
        cc8 = const.tile([128, ND * 2 * K], F8)
        nc.sync.dma_start(out=cc8[:], in_=cc8_in[:])
        ncsq_row = const.tile([1, K], F32)
        nc.sync.dma_start(out=ncsq_row[:], in_=nrow_in[:])
        ncsq_b = const.tile([128, K], F32)
        nc.gpsimd.partition_broadcast(ncsq_b[:], ncsq_row[:])

        identf = const.tile([128, 128], F32)
        make_identity(nc, identf[:])

        idx8 = const.tile([128, n_tiles * 8], U32, tag="idx8")
        fbuf = const.tile([128, n_tiles], F32)

        def prep(rt):
            """Load + transpose + derive per-tile operands (pipelined)."""
            ftile = work.tile([128, D], F32, tag="ftile")
            nc.sync.dma_start(out=ftile[:], in_=feat[rt * 128:(rt + 1) * 128, :])

            # one fp32 PE transpose; hi/lo split in the transposed domain
            tpf = psA.tile([128, ND * 128], F32, tag="tpf")
            for d in range(ND):
                nc.tensor.transpose(tpf[:, d * 128:(d + 1) * 128],
                                    ftile[:, d * 128:(d + 1) * 128], identf[:])
            fhl16T = tps.tile([128, 2 * ND * 128], F16, tag="fhl16T")
            nc.scalar.copy(out=fhl16T[:, 0:512], in_=tpf[:])
            # PSUM source: must be DVE (gpsimd cannot access PSUM)
            nc.vector.tensor_tensor(out=fhl16T[:, 512:1024], in0=tpf[:],
                                    in1=fhl16T[:, 0:512],
                                    op=mybir.AluOpType.subtract)

            # fp8 stationary pair tile [a8T (512) | b8T (512)]
            ab8T = tps.tile([128, 2 * ND * 128], F8, tag="ab8T")
            nc.scalar.copy(out=ab8T[:, 0:512], in_=fhl16T[:, 0:512])
            nc.scalar.activation(out=ab8T[:, 512:1024], in_=fhl16T[:, 512:1024],
                                 func=COPY, scale=BSCALE)
            return fhl16T, ab8T

        def compute(rt, fhl16T, ab8T):
            mp = psM.tile([128, K], F32, tag="mp")
            for d in range(ND):  # main f16 pass
                for kh in range(2):
                    nc.tensor.matmul(
                        mp[:, kh * 512:(kh + 1) * 512],
                        lhsT=fhl16T[:, d * 128:(d + 1) * 128],
                        rhs=cmain[:, d * K + kh * 512:d * K + (kh + 1) * 512],
                        start=(d == 0), stop=False)
            ab2 = ab8T[:].rearrange("p (two x) -> p two x", two=2)
            for d in range(ND):  # fp8 DoubleRow corrections
                for kh in range(2):
                    nc.tensor.matmul(
                        mp[:, kh * 512:(kh + 1) * 512],
                        lhsT=ab2[:, :, d * 128:(d + 1) * 128],
                        rhs=cc8[:, (d * 2 + kh) * K:(d * 2 + kh + 1) * K]
                            .rearrange("p (two k) -> p two k", two=2),
                        perf_mode=DRMODE,
                        start=False, stop=(d == ND - 1))
            # ncsq bias applied on the DVE (keeps the PE stream minimal);
            # argmax runs on the biased SBUF copy
            m_b = work.tile([128, K], F32, tag="m_b")
            nc.vector.tensor_tensor(out=m_b[:], in0=mp[:], in1=ncsq_b[:],
                                    op=mybir.AluOpType.add)
            mv = red.tile([128, 8], F32, tag="mv")
            nc.vector.max(mv[:], m_b[:])
            nc.vector.max_index(idx8[:, rt * 8:(rt + 1) * 8], mv[:], m_b[:])

        loop_ctx = tc.For_i(0, repeat, 1) if repeat > 1 else nullcontext()
        with loop_ctx:
            pending = prep(0)
            for rt in range(n_tiles):
                nxt = prep(rt + 1) if rt + 1 < n_tiles else None
                compute(rt, *pending)
                pending = nxt

        nc.vector.tensor_copy(out=fbuf[:], in_=idx8[:, 0:n_tiles * 8:8])
        nc.sync.dma_start(out=out[:, 0].rearrange("(t p) -> p t", p=128),
                          in_=fbuf[:])

    if dedup:
        n = _dedup_ldweights(nc)
        if os.environ.get("KM_DEBUG"):
            print(f"dedup_ldweights removed {n}")
    nc.finalize()
    return nc


def prep_c_side(centroids: np.ndarray):
    """Host prep of all centroid-side operands in final SBUF layouts."""
    c = centroids.astype(np.float32)
    c2 = (2.0 * c).astype(np.float32)
    C16s = (c2 * PSCALE).astype(np.float16)           # [D, K], scale 2^6
    clz = c2 - C16s.astype(np.float32) / PSCALE       # exact residual
    CL8 = (clz * PSCALE).astype(E4NP)                 # pairs a8 (scale 1)
    CH8 = (c2 * (PSCALE / BSCALE)).astype(E4NP)       # pairs b8 (scale 2^11)
    nrow = (-(c.astype(np.float64) ** 2).sum(0, keepdims=True)
            * PSCALE).astype(np.float32)                # [1, K], scale 2^6

    # cmain layout [128, ND*K]: partition p holds C16s[a*128+p, k] at col a*K+k
    cmain = np.ascontiguousarray(
        C16s.reshape(ND, 128, K).transpose(1, 0, 2).reshape(128, ND * K))
    # cc8 layout [128, ND*2*K]: per (d, kh) a 1024-col block [CL8(512)|CH8(512)]
    CL8r = CL8.reshape(ND, 128, 2, 512)               # [d, p, kh, k]
    CH8r = CH8.reshape(ND, 128, 2, 512)
    blk = np.stack([CL8r, CH8r], axis=3)              # [d, p, kh, two, k]
    cc8 = np.ascontiguousarray(
        blk.transpose(1, 0, 2, 3, 4).reshape(128, ND * 2 * K))
    return cmain, cc8, nrow


def prep_inputs(features: np.ndarray, centroids: np.ndarray):
    features = np.ascontiguousarray(np.asarray(features, dtype=np.float32))
    cmain, cc8, nrow = prep_c_side(np.asarray(centroids))
    return [
        {
            "features": features[cidx * N_PER_CORE:(cidx + 1) * N_PER_CORE],
            "cmain": cmain,
            "cc8": cc8,
            "nrow": nrow,
        }
        for cidx in range(N_CORES)
    ]


def _get_nc():
    if "nc" not in _cached:
        _cached["nc"] = build_bass()
    return _cached["nc"]


def kernel(features: np.ndarray, centroids: np.ndarray) -> np.ndarray:
    from concourse.bass_utils import run_bass_kernel_spmd
    in_maps = prep_inputs(features, centroids)
    nc = _get_nc()
    res = run_bass_kernel_spmd(nc, in_maps, list(range(N_CORES))).results
    out = np.concatenate([res[c]["out"] for c in range(N_CORES)], axis=0)
    return out.astype(np.float32)


def _self_test():
    rng = np.random.default_rng(0)
    f = rng.standard_normal((N, D)).astype(np.float32)
    c = rng.standard_normal((D, K)).astype(np.float32)
    out = kernel(f, c)
    x = f @ c
    ref = (-2 * x + (c * c).sum(0)).argmin(1)
    print("mismatch:", (out[:, 0] != ref).sum(), "/", N)


if __name__ == "__main__":
    _self_test()
